# revision 23
# baseline (speedup 1.0000x reference)

"""Causal attention (no head split) on 8 trn2 NeuronCores.

Reference computation (per batch b):
    q = x @ Wq^T ; k = x @ Wk^T ; v = x @ Wv^T          (nn.Linear convention)
    wei = softmax(mask(q @ k^T / sqrt(C)))               (causal)
    out = wei @ v

Algebraic restructuring (K and V are never materialized):
    S   = q k^T = x (Wq^T Wk) x^T = x M x^T     with M precomputed on host
    out = wei v = (wei x) Wv^T, i.e. O^T = Wv (x^T wei^T) = Wv H
so the device only computes:
    G^T = M^T xq^T                  (one projection of this core's queries)
    S^T[s,t] = x^T(lhsT) G^T(rhs)   (contract over C)
    P^T = exp(S^T / 32) * mask ; rowsum[t] += ones^T P^T
    H[c,t] += x(lhsT) P^T(rhs)      (contract over s, accumulated in SBUF)
    O^T = Wv^T-projection of H      (once per finished query strip)
Final softmax normalization (divide by rowsum) happens on the host.

Sharding: 2 cores per batch (B=4). Queries split into eight 256-row strips;
role A takes strips {0,2,4,6} (rows [512j,512j+256)), role B {1,3,5,7}.
Every core runs the IDENTICAL instruction stream (single SPMD NEFF); role
differences are carried entirely by input data (query columns + mask tiles).

All streams are bf16 (keeps FWL enabled on the PE -> hidden weight loads,
and halves DMA). Accumulation (PSUM, H, rowsum) stays fp32.

Keys are processed in PAIRS of 256-chunks (512 keys per visit): H partial
sums accumulate across the whole pair inside PSUM (start/stop over 4
ss-subtiles), halving the vector tensor_add traffic into H, the rowsum
updates, and the number of stream DMAs. The causal structure is pair-
aligned: strip j attends exactly pairs 0..j, and the diagonal pair's mask
is a single [P, 4, SW] data tile per role.

Every DRAM tensor is PRE-LAYOUTED on the host into per-partition-contiguous
DMA order (partition-row-major, 2-16KB contiguous runs), so each dma_start
is a handful of large descriptors: issue cost and DRAM efficiency both
matter — 256B-run strided DMAs starve the G phase.

Scheduling: the G phase is DMA-latency-bound at kernel start, so pair-0
attention visits are interleaved between G(strips); each visit is split
into S (S-matmuls + exp) and RH (rowsum + H matmuls) and RH is emitted one
visit behind S, so the PE never waits on the exp->mask chain. DMA issue is
spread over sync (weights + xt), scalar (more weights), gpsimd (xq, xna,
output drain).
"""
import os
import numpy as np
import ml_dtypes

import concourse.bass as bass
from concourse import bacc
import concourse.mybir as mybir
from concourse.tile import TileContext
from concourse import bass_utils

B, T, C = 4, 2048, 1024
P = 128
CS = C // P          # 8 contraction subtiles
NPAIR = T // 512     # 4 kv chunk-pairs of 512
QS = 4               # query strips per core
SW = 256             # strip width
SCALE = 1.0 / np.sqrt(C)  # 1/32

BF16 = mybir.dt.bfloat16
F32 = mybir.dt.float32
NPBF16 = ml_dtypes.bfloat16


def build():
    nc = bacc.Bacc(trn_type="TRN2", name="causal_attn")
    # host-layouted, partition-row-major (see make_in_maps)
    xtD = nc.dram_tensor("xtD", [P, NPAIR * CS * 512], BF16, kind="ExternalInput")
    xnD = nc.dram_tensor("xnD", [P, NPAIR * 4 * C], BF16, kind="ExternalInput")
    xqD = nc.dram_tensor("xqD", [P, QS * CS * SW], BF16, kind="ExternalInput")
    wmD = nc.dram_tensor("wmD", [CS * P, CS * P], BF16, kind="ExternalInput")
    wvD = nc.dram_tensor("wvD", [P, CS * C], BF16, kind="ExternalInput")
    masks = nc.dram_tensor("masks", [P, 4 * SW], BF16, kind="ExternalInput")
    ones = nc.dram_tensor("ones", [P, 1], BF16, kind="ExternalInput")
    outT = nc.dram_tensor("outT", [C, QS * SW], BF16, kind="ExternalOutput")
    rows = nc.dram_tensor("rows", [1, QS * SW], F32, kind="ExternalOutput")

    xt_r = xtD.rearrange("p (pp cs t) -> p pp cs t", pp=NPAIR, cs=CS)
    xn_r = xnD.rearrange("p (pp ss c) -> p pp ss c", pp=NPAIR, ss=4)
    xq_r = xqD.rearrange("p (jj cs t) -> p jj cs t", jj=QS, cs=CS)
    wm_r3 = wmD.rearrange("(ds p) (cs dd) -> p ds cs dd", p=P, cs=CS)
    wv_r = wvD.rearrange("p (cs d) -> p cs d", cs=CS)
    msk_r = masks.rearrange("p (ss t) -> p ss t", ss=4)
    outT_r = outT.rearrange("(ds p) t -> p ds t", p=P)
    rows_r = rows.rearrange("p (a b) -> p a b", a=QS)

    with TileContext(nc) as tc:
        with tc.tile_pool(name="keep", bufs=1) as keep, \
             tc.tile_pool(name="wpool", bufs=2) as wpool, \
             tc.tile_pool(name="qpool", bufs=1) as qpool, \
             tc.tile_pool(name="stream", bufs=4) as stream, \
             tc.tile_pool(name="hrpool", bufs=2) as hrpool, \
             tc.tile_pool(name="ppool", bufs=3) as ppool, \
             tc.tile_pool(name="psA", bufs=2, space="PSUM") as psA, \
             tc.tile_pool(name="psS", bufs=3, space="PSUM") as psS, \
             tc.tile_pool(name="psO", bufs=2, space="PSUM") as psO, \
             tc.tile_pool(name="psR", bufs=1, space="PSUM") as psR:

            gT = keep.tile([P, CS, QS * SW], BF16, tag="gT")   # G^T  16KB/part
            hh = keep.tile([P, CS, QS * SW], F32, tag="hh")    # H    32KB/part
            msk = keep.tile([P, 4, SW], BF16, tag="msk")
            ones_t = keep.tile([P, 1], BF16, tag="ones")
            rowsum = keep.tile([1, QS, SW], F32, tag="rowsum")

            # ---- prologue DMA. DMA transfers occupy the issuing engine's
            # queue, so: scalar gets NO DMAs (it drains gT/exp), sync carries
            # weights + xt streams, gpsimd carries xq/xna + output drain.
            # wq is [p][ds][cs][dd] so each ds-block DMA is contiguous 2KB
            # runs on both sides (descriptor merging needs this).
            wq = wpool.tile([P, CS, CS, P], BF16, tag="w")
            nc.sync.dma_start(wq[:, 0, 0:2], wm_r3[:, 0, 0:2])
            nc.sync.dma_start(wq[:, 0, 2:8], wm_r3[:, 0, 2:8])
            for ds in range(1, CS):
                nc.sync.dma_start(wq[:, ds], wm_r3[:, ds])

            # xqa gets absolute priority on gpsimd: the ds-major G phase
            # consumes all four strips' queries within its first ds-block.
            xqa = qpool.tile([P, QS, CS, SW], BF16, tag="xq")
            jsl = lambda j: slice(j * SW, (j + 1) * SW)
            nc.gpsimd.dma_start(xqa[:, 0, 0:4], xq_r[:, 0, 0:4])
            nc.gpsimd.dma_start(xqa[:, 0, 4:8], xq_r[:, 0, 4:8])
            nc.gpsimd.dma_start(xqa[:, 1], xq_r[:, 1])
            nc.gpsimd.dma_start(xqa[:, 2], xq_r[:, 2])
            nc.gpsimd.dma_start(xqa[:, 3], xq_r[:, 3])
            nc.gpsimd.dma_start(ones_t[:], ones[:])
            nc.gpsimd.dma_start(msk[:], msk_r[:])

            def stream_pair(p, engine):
                xt2 = stream.tile([P, CS, 512], BF16, tag="xt")
                engine.dma_start(xt2[:], xt_r[:, p])
                xna2 = stream.tile([P, 4, C], BF16, tag="xn")
                nc.gpsimd.dma_start(xna2[:], xn_r[:, p])
                return xt2, xna2

            pair0 = stream_pair(0, nc.sync)
            wv = wpool.tile([P, CS, C], BF16, tag="w")
            nc.sync.dma_start(wv[:], wv_r[:])

            def G_all():
                # ds-major: each 256KB weight block feeds 32 matmuls (3.5us),
                # so the wm stream needs only ~73GB/s instead of racing the
                # whole 2MB into the first 7us (strip-major starved the PE).
                for ds in range(CS):
                    for j in range(QS):
                        pq = psA.tile([P, SW], F32, tag="prod")
                        for cs in range(CS):
                            nc.tensor.matmul(
                                pq[:], wq[:, ds, cs], xqa[:, j, cs],
                                start=(cs == 0), stop=(cs == CS - 1))
                        nc.scalar.copy(gT[:, ds, jsl(j)], pq[:])

            def S(p, j, xt2, xna2):
                """S^T matmuls + exp (+ diagonal mask) for visit (pair p, strip j)."""
                tsl = jsl(j)
                pT = ppool.tile([P, 4, SW], BF16, tag="pT")
                for half in range(2):
                    st = psS.tile([P, 2, SW], F32, tag="st")
                    for sl in range(2):
                        ss = 2 * half + sl
                        for cs in range(CS):
                            nc.tensor.matmul(
                                st[:, sl], xt2[:, cs, ss * P:(ss + 1) * P],
                                gT[:, cs, tsl],
                                start=(cs == 0), stop=(cs == CS - 1))
                    nc.scalar.activation(
                        pT[:, 2 * half:2 * half + 2], st[:],
                        mybir.ActivationFunctionType.Exp, scale=float(SCALE))
                if p == j:     # diagonal pair: causal mask via data tile
                    nc.vector.tensor_mul(pT[:], pT[:], msk[:])
                return (p, j, pT, xna2)

            def RH(v):
                """rowsum + H accumulation for a visit whose S/exp is in flight."""
                p, j, pT, xna2 = v
                tsl = jsl(j)
                rw = psR.tile([1, SW], F32, tag="rw")
                for ss in range(4):
                    nc.tensor.matmul(
                        rw[:], ones_t[:], pT[:, ss],
                        start=(ss == 0), stop=(ss == 3))
                if p == 0:
                    nc.vector.tensor_copy(rowsum[:, j], rw[:])
                else:
                    nc.vector.tensor_add(rowsum[:, j], rowsum[:, j], rw[:])

                # H[c,t] += x(lhsT) @ P^T; whole 512-key pair accumulates in
                # PSUM before a single vector add per c-quarter.
                for q4 in range(4):
                    po = psO.tile([P, 2, SW], F32, tag="po")
                    for i in range(2):
                        cs4 = 2 * q4 + i
                        for ss in range(4):
                            nc.tensor.matmul(
                                po[:, i], xna2[:, ss, cs4 * P:(cs4 + 1) * P],
                                pT[:, ss],
                                start=(ss == 0), stop=(ss == 3))
                    hsl = hh[:, 2 * q4:2 * q4 + 2, tsl]
                    if p == 0:
                        nc.vector.tensor_copy(hsl, po[:])
                    else:
                        nc.vector.tensor_add(hsl, hsl, po[:])

            def proj(j, tail=False):
                # strip j's H is complete: O^T = Wv^T-projection, drained
                # (vector cast + outT DMA on the gpsimd queue). The very
                # last strip flushes per-ds on BOTH queues so the two DMA
                # drains at kernel end each wait on half as much data.
                tsl = jsl(j)
                hr = hrpool.tile([P, CS, SW], BF16, tag="hr")
                for q2 in range(2):
                    nc.scalar.copy(hr[:, 4 * q2:4 * q2 + 4],
                                   hh[:, 4 * q2:4 * q2 + 4, tsl])
                ost = hrpool.tile([P, CS, SW], BF16, tag="ost")
                for ds in range(CS):
                    pf = psA.tile([P, SW], F32, tag="prod")
                    for cs in range(CS):
                        nc.tensor.matmul(
                            pf[:], wv[:, cs, ds * P:(ds + 1) * P], hr[:, cs],
                            start=(cs == 0), stop=(cs == CS - 1))
                    nc.vector.tensor_copy(ost[:, ds], pf[:])
                    if tail:
                        eng = nc.sync if ds % 2 == 0 else nc.gpsimd
                        eng.dma_start(outT_r[:, ds, tsl], ost[:, ds])
                    elif ds % 2 == 1:
                        nc.gpsimd.dma_start(
                            outT_r[:, ds - 1:ds + 1, tsl],
                            ost[:, ds - 1:ds + 1])

            # ---- emission: visits in pair order [0,1,3,2] (strips 2 and 3
            # finish together at the end so their projections interleave);
            # RH lags S by one visit; G interleaved with pair-0 work.
            G_all()
            v0 = S(0, 0, *pair0)
            v1 = S(0, 1, *pair0)
            RH(v0)
            pair1 = stream_pair(1, nc.sync)
            v2 = S(0, 2, *pair0)
            RH(v1)
            proj(0)
            v3 = S(0, 3, *pair0)
            RH(v2)
            pair3 = stream_pair(3, nc.sync)
            v4 = S(1, 1, *pair1)
            RH(v3)
            v5 = S(1, 2, *pair1)
            RH(v4)
            proj(1)
            pair2 = stream_pair(2, nc.sync)
            v6 = S(1, 3, *pair1)
            RH(v5)
            v7 = S(3, 3, *pair3)
            RH(v6)
            v8 = S(2, 2, *pair2)
            RH(v7)
            v9 = S(2, 3, *pair2)
            RH(v8)
            RH(v9)
            # rows rides the idle scalar queue so the tail barrier never
            # waits on it behind the outT flush.
            nc.scalar.dma_start(rows_r[:], rowsum[:])
            proj(2)
            proj(3, tail=True)

    nc.compile()
    return nc


_NC = None


def _get_nc():
    global _NC
    if _NC is None:
        _NC = build()
    return _NC


def make_in_maps(x, Wq, Wk, Wv):
    x = np.asarray(x, dtype=np.float32)
    wq64 = np.asarray(Wq, np.float64)
    wk64 = np.asarray(Wk, np.float64)
    wm = (wq64.T @ wk64).astype(NPBF16)                      # M = Wq^T Wk [c',c]
    # wmD[ds*128+p, cs*128+dd] = M[cs*128+p, ds*128+dd]  (per-partition 2KB runs)
    wmD = np.ascontiguousarray(
        wm.reshape(CS, P, CS, P).transpose(2, 1, 0, 3).reshape(CS * P, CS * P))
    # wvD[p, cs*1024+d] = Wv^T[cs*128+p, d] = Wv[d, cs*128+p]
    wvT = np.asarray(Wv, np.float32).T.astype(NPBF16)        # [c, d]
    wvD = np.ascontiguousarray(
        wvT.reshape(CS, P, C).transpose(1, 0, 2).reshape(P, CS * C))
    ones = np.ones((P, 1), NPBF16)

    # mask tiles [p, ss*SW+t] over a diagonal 512-key pair: visible iff
    # (key offset within pair) <= (query offset within strip) + 256*role
    s_idx = (np.arange(2)[:, None, None] * P + np.arange(P)[None, :, None])
    tri = (s_idx <= np.arange(SW)[None, None, :]).astype(np.float32)
    tri = np.ascontiguousarray(tri.transpose(1, 0, 2))
    zeros = np.zeros((P, 2, SW), np.float32)
    ones2 = np.ones((P, 2, SW), np.float32)
    mask_A = np.concatenate([tri, zeros], axis=1).reshape(P, 4 * SW).astype(NPBF16)
    mask_B = np.concatenate([ones2, tri], axis=1).reshape(P, 4 * SW).astype(NPBF16)

    in_maps = []
    for b in range(B):
        xb = x[b].astype(NPBF16)                             # [T, C]
        xTb = np.ascontiguousarray(xb.T)                     # [C, T]
        # xtD[p, pp*CS*512 + cs*512 + k] = x^T[cs*128+p, pp*512+k]
        xtD = np.ascontiguousarray(
            xTb.reshape(CS, P, NPAIR, 512).transpose(1, 2, 0, 3).reshape(P, -1))
        # xnD[p, pp*4*C + ss*C + c] = x[pp*512 + ss*128 + p, c]
        xnD = np.ascontiguousarray(
            xb.reshape(NPAIR, 4, P, C).transpose(2, 0, 1, 3).reshape(P, -1))
        for role in range(2):
            cols = np.concatenate(
                [np.arange(512 * j + SW * role, 512 * j + SW * role + SW)
                 for j in range(QS)])
            xqT = xTb[:, cols]                               # [C, 1024]
            # xqD[p, j*CS*SW + cs*SW + t] = xqT[cs*128+p, j*SW+t]
            xqD = np.ascontiguousarray(
                xqT.reshape(CS, P, QS, SW).transpose(1, 2, 0, 3).reshape(P, -1))
            in_maps.append({
                "xtD": xtD,
                "xnD": xnD,
                "xqD": xqD,
                "wmD": wmD, "wvD": wvD,
                "masks": mask_A if role == 0 else mask_B,
                "ones": ones,
            })
    return in_maps


def assemble(results):
    out = np.empty((B, T, C), np.float32)
    for core in range(8):
        b, role = divmod(core, 2)
        oT = np.asarray(results[core]["outT"]).astype(np.float32)  # [C, 1024]
        rsum = np.asarray(results[core]["rows"]).reshape(QS * SW)
        o = oT.T / rsum[:, None]
        for j in range(QS):
            r0 = 512 * j + SW * role
            out[b, r0:r0 + SW] = o[j * SW:(j + 1) * SW]
    return out


def kernel(x, Wq, Wk, Wv):
    nc = _get_nc()
    in_maps = make_in_maps(x, Wq, Wk, Wv)
    res = bass_utils.run_bass_kernel_spmd(nc, in_maps, core_ids=list(range(8)))
    return assemble(res.results)


def _install_trace_shim():
    """Provide antenv.axon_hooks (absent in this image) so trace=True works."""
    import sys
    import types
    if "antenv.axon_hooks" in sys.modules:
        return
    hook_box = [None]
    mod = types.ModuleType("antenv.axon_hooks")
    mod.set_axon_ntff_profile_hook = lambda h: hook_box.__setitem__(0, h)
    mod.get_axon_ntff_profile_hook = lambda: hook_box[0]
    import antenv
    sys.modules["antenv.axon_hooks"] = mod
    antenv.axon_hooks = mod
    try:
        from trn_agent_boot.trn_boot import _ntff_profile_via_ctypes
        mod.set_axon_ntff_profile_hook(
            _ntff_profile_via_ctypes("/opt/axon/libaxon_pjrt.so"))
    except Exception:
        pass


def run_traced(x, Wq, Wk, Wv):
    """Like kernel() but with NTFF tracing; returns (out, BassKernelResults)."""
    _install_trace_shim()
    nc = _get_nc()
    in_maps = make_in_maps(x, Wq, Wk, Wv)
    res = bass_utils.run_bass_kernel_spmd(
        nc, in_maps, core_ids=list(range(8)), trace=True,
        trace_cores=list(range(8)))
    return assemble(res.results), res
